# revision 14
# baseline (speedup 1.0000x reference)
"""Two-layer GraphSAGE (mean aggregation) on 8 Trainium2 NeuronCores.

Strategy (dst-partitioning per the hint), v5:
- Nodes partitioned by destination across 8 cores (12500 each, padded to
  12544 = 98*128 rows per core). Gather tables use an A/B relayout of the
  padded id space: A = each core's first 8192 rows (65536 total), B = the
  remaining 4352 (34816 total). Windows of <=32768 rows (2 over A, 2 over
  B) keep dma_gather indices in int16, and the A/B split lets the
  inter-layer AllGather run as two collectives: CC-A is issued as soon as
  layer-1 tiles 0..63 are done, so layer-2's A-window gathers overlap
  CC-B and layer-1's tail.
- x[src] rows are fetched in bf16 with batched SWDGE dma_gather (1024
  descriptors per instruction — the ucode ring limit — rotating across
  the 4 SWDGE queues).
- Aggregation per 128-dst tile: indicator matmuls on the PE. Indicators
  (0/1) are built in one DVE is_equal per (group, window) span; the 1/deg
  mean scaling is fused into the PSUM drain against a host-shipped
  broadcast reciprocal table.
- Downstream stays in transposed [feat, node] layout: hT = W_l^T @ aggT
  + W_r^T @ xT, so bias+relu+PSUM-drain is one Activation op and
  layer-2's self term reuses layer-1's resident hT tile. Row-layout h is
  produced with PE transposes and one batched DMA per tile-group.

kernel(**inputs) -> np.ndarray takes FULL inputs, returns FULL output.
"""

import os

import numpy as np

P = 128
NCORES = 8
NPC = 12500              # nodes per core
TPC = 98                 # 128-node tiles per core
NPC_PAD = TPC * P        # 12544
NALL = NCORES * NPC_PAD  # 100352
ASPL = 8192              # per-core rows in the A half (64 tiles)
BSPL = NPC_PAD - ASPL    # 4352
NA = NCORES * ASPL       # 65536
NB = NCORES * BSPL       # 34816
TSPL = ASPL // P         # tile index of the A/B boundary (64)
NWIN = 4
# window w -> (table, base within table); tables: 0 = A, 1 = B
WTAB = [0, 0, 1, 1]
WBASE_T = [0, 32768, 0, 32768]
WSIZE = [32768, 32768, 32768, NB - 32768]
TPG = 12                 # tiles per group
DENSE_B = 4              # tiles per dense-matmul batch


def _newpos(srcpad):
    """Map padded global id (core*NPC_PAD + local) to A/B-relaid position:
    A-part ids [0, NA), B-part ids [NA, NA+NB)."""
    core = srcpad // NPC_PAD
    local = srcpad % NPC_PAD
    a = local < ASPL
    return np.where(
        a, core * ASPL + local, NA + core * BSPL + (local - ASPL)
    )


def _prep_edges(edge_index: np.ndarray, n_nodes: int):
    src = edge_index[0].astype(np.int64)
    dst = edge_index[1].astype(np.int64)
    srcpad = (src // NPC) * NPC_PAD + (src % NPC)
    pos = _newpos(srcpad)  # 0..NA+NB
    rng = pos >> 15        # window 0..3 (NA=2*32768; window 3 = B tail)
    core = dst // NPC
    loc = dst % NPC
    tl = loc // P
    off = loc % P

    key = (core * TPC + tl) * NWIN + rng
    cnt = np.bincount(key, minlength=NCORES * TPC * NWIN).reshape(
        NCORES, TPC, NWIN
    )
    ch = -(-cnt.max(axis=0) // P)  # [TPC, NWIN]
    assert ch.sum(axis=1).min() >= 1

    colof = np.zeros((TPC, NWIN), np.int64)
    groups = []  # (t0, t1, gc0, gc1, spans[r] = (c0, c1))
    c = 0
    for g0 in range(0, TPC, TPG):
        t0, t1 = g0, min(g0 + TPG, TPC)
        gc0 = c
        spans = []
        for r in range(NWIN):
            rc0 = c
            for t in range(t0, t1):
                colof[t, r] = c
                c += ch[t, r]
            spans.append((rc0, c))
        groups.append((t0, t1, gc0, c, spans))
    ncols = int(c)
    S = ncols * P

    order = np.argsort(key, kind="stable")
    sk = key[order]
    first = np.r_[True, sk[1:] != sk[:-1]]
    idx_of_first = np.where(first)[0]
    grp_id = np.cumsum(first) - 1
    rank = np.arange(len(sk)) - idx_of_first[grp_id]
    slot = colof[tl[order], rng[order]] * P + rank

    idxflat = np.zeros((NCORES, S), np.int16)
    edstflat = np.full((NCORES, S), -1.0, np.float32)
    wb = np.asarray([0, 32768, NA, NA + 32768], np.int64)
    idxval = (pos - wb[rng]).astype(np.int16)
    co = core[order]
    idxflat[co, slot] = idxval[order]
    edstflat[co, slot] = off[order].astype(np.float32)

    deg = np.bincount(dst, minlength=n_nodes).astype(np.float64)
    recip = (1.0 / np.maximum(deg, 1.0)).astype(np.float32)

    idx16 = np.ascontiguousarray(
        np.tile(idxflat.reshape(NCORES, S // 16, 16).transpose(0, 2, 1), (1, 8, 1))
    )
    edst = np.ascontiguousarray(
        edstflat.reshape(NCORES, ncols, P).transpose(0, 2, 1)
    )
    return ch, colof, ncols, groups, idx16, edst, recip


def _build_program(ncols, groups, tile_cols, tile_colsA, tile_colsB, gcmax, gmaxc):
    from concourse import bacc, library_config, mybir, tile

    f32 = mybir.dt.float32
    bf16 = mybir.dt.bfloat16
    i16 = mybir.dt.int16

    nc = bacc.Bacc(
        "TRN2",
        target_bir_lowering=False,
        debug=False,
        num_devices=NCORES,
        num_swdge_queues=4,
    )

    # x gather table in A/B-relaid row order: rows [0,NA) then [NA,NA+NB)
    xg_d = nc.declare_dram_parameter("xg", [NA + NB, P], bf16, isOutput=False)
    xownT_d = nc.declare_dram_parameter("xownT", [P, NPC_PAD], bf16,
                                        isOutput=False)
    idx_d = nc.declare_dram_parameter("idx16", [P, (ncols * P) // 16], i16,
                                      isOutput=False)
    edst_d = nc.declare_dram_parameter("edst", [P, ncols], bf16, isOutput=False)
    recipb_d = nc.declare_dram_parameter("recipb", [P, NPC_PAD], bf16,
                                         isOutput=False)
    wl1_d = nc.declare_dram_parameter("wl1", [P, P], bf16, isOutput=False)
    wr1_d = nc.declare_dram_parameter("wr1", [P, P], bf16, isOutput=False)
    wl2_d = nc.declare_dram_parameter("wl2", [P, P], bf16, isOutput=False)
    wr2_d = nc.declare_dram_parameter("wr2", [P, P], bf16, isOutput=False)
    bias1_d = nc.declare_dram_parameter("bias1", [P, 1], f32, isOutput=False)
    bias2_d = nc.declare_dram_parameter("bias2", [P, 1], f32, isOutput=False)
    iota_d = nc.declare_dram_parameter("iota", [P, P], bf16, isOutput=False)
    ident_d = nc.declare_dram_parameter("ident", [P, P], bf16, isOutput=False)
    out_d = nc.declare_dram_parameter("out", [NPC_PAD, P], f32, isOutput=True)

    is_eq = mybir.AluOpType.is_equal
    mult = mybir.AluOpType.mult

    with tile.TileContext(nc) as tc:
        with (
            tc.tile_pool(name="const", bufs=1) as cpool,
            tc.tile_pool(name="big", bufs=1) as bigpool,
            tc.tile_pool(name="gath", bufs=2) as gpool,
            tc.tile_pool(name="indp", bufs=2) as ipool,
            tc.tile_pool(name="agg", bufs=2) as apool,
            tc.tile_pool(name="row", bufs=2) as rpool,
            tc.tile_pool(name="psacc", bufs=3, space="PSUM") as ps_a,
            tc.tile_pool(name="psh", bufs=2, space="PSUM") as ps_h,
            tc.tile_pool(name="pst", bufs=2, space="PSUM") as ps_t,
            tc.tile_pool(name="dram", bufs=1, space="DRAM") as dpool,
        ):
            nc.gpsimd.load_library(library_config.mlp)

            def load_const(dram_ap, shape, dtype, name):
                t = cpool.tile(shape, dtype, name=name)
                nc.sync.dma_start(out=t[:], in_=dram_ap)
                return t

            wl1 = load_const(wl1_d[:], [P, P], bf16, "wl1")
            wr1 = load_const(wr1_d[:], [P, P], bf16, "wr1")
            wl2 = load_const(wl2_d[:], [P, P], bf16, "wl2")
            wr2 = load_const(wr2_d[:], [P, P], bf16, "wr2")
            bias1 = load_const(bias1_d[:], [P, 1], f32, "bias1")
            bias2 = load_const(bias2_d[:], [P, 1], f32, "bias2")
            iota = load_const(iota_d[:], [P, P], bf16, "iota")
            ident = load_const(ident_d[:], [P, P], bf16, "ident")
            xownT = bigpool.tile([P, NPC_PAD], bf16, tag="big", name="xownT")
            nc.sync.dma_start(out=xownT[:], in_=xownT_d[:])
            idx16 = load_const(idx_d[:], [P, (ncols * P) // 16], i16, "idx16")
            edst = load_const(edst_d[:], [P, ncols], bf16, "edst")
            recipb = load_const(recipb_d[:], [P, NPC_PAD], bf16, "recipb")

            hT = cpool.tile([P, NPC_PAD], bf16, name="hT")

            hbA = dpool.tile([ASPL, P], bf16, name="hbA")
            hbB = dpool.tile([BSPL, P], bf16, name="hbB")
            hfA = dpool.tile([NCORES, ASPL, P], bf16, name="hfA",
                             addr_space="Shared")
            hfB = dpool.tile([NCORES, BSPL, P], bf16, name="hfB",
                             addr_space="Shared")
            hfA2 = hfA[:].rearrange("c n d -> (c n) d")
            hfB2 = hfB[:].rearrange("c n d -> (c n) d")

            qctr = [0]

            def gather_src(tables, r):
                t2d = tables[WTAB[r]]
                b = WBASE_T[r]
                return t2d[b : b + WSIZE[r], :]

            def layer(tables, wins, mode, selfT, soff_base, wl, wr, bias,
                      relu, hTdst_fn, row_write, aggP=None,
                      tcols_fn=None):
                for gi, (t0, t1, gc0, gc1, spans) in enumerate(groups):
                    nt = t1 - t0
                    gbuf = gpool.tile([P, gcmax, P], bf16, tag="g")
                    ibuf = ipool.tile([P, gcmax, P], bf16, tag="i")
                    for r in wins:
                        c0, c1 = spans[r]
                        if c1 == c0:
                            continue
                        for s0 in range(c0, c1, gmaxc):
                            s1 = min(s0 + gmaxc, c1)
                            n_idx = (s1 - s0) * P
                            nc.gpsimd.dma_gather(
                                gbuf[:, s0 - gc0 : s1 - gc0, :],
                                gather_src(tables, r),
                                idx16[:, s0 * 8 : s1 * 8],
                                n_idx,
                                n_idx,
                                P,
                                queue_num=qctr[0] % 4,
                            )
                            qctr[0] += 1
                        nc.vector.tensor_tensor(
                            out=ibuf[:, c0 - gc0 : c1 - gc0, :],
                            in0=edst[:, c0:c1, None].to_broadcast(
                                [P, c1 - c0, P]
                            ),
                            in1=iota[:, None, :].to_broadcast([P, c1 - c0, P]),
                            op=is_eq,
                        )
                    agg = (None if mode == "partial" else
                           apool.tile([P, nt, P], bf16, tag="agg", name="agg"))
                    for ti, t in enumerate(range(t0, t1)):
                        cols = tcols_fn(t)
                        acc = ps_a.tile([P, P], f32, tag="acc")
                        last = len(cols) - 1
                        for ci, col in enumerate(cols):
                            nc.tensor.matmul(
                                out=acc[:],
                                lhsT=gbuf[:, col - gc0, :],
                                rhs=ibuf[:, col - gc0, :],
                                start=(ci == 0),
                                stop=(ci == last),
                            )
                        if mode == "partial":
                            nc.vector.tensor_copy(
                                out=aggP[:, t * P : (t + 1) * P], in_=acc[:]
                            )
                            continue
                        if mode == "finish":
                            psum = apool.tile([P, P], bf16, tag="psum",
                                              name="psum")
                            nc.vector.tensor_tensor(
                                out=psum[:],
                                in0=acc[:],
                                in1=aggP[:, t * P : (t + 1) * P],
                                op=mybir.AluOpType.add,
                            )
                            nc.vector.tensor_tensor(
                                out=agg[:, ti, :],
                                in0=psum[:],
                                in1=recipb[:, t * P : (t + 1) * P],
                                op=mult,
                            )
                        else:
                            nc.vector.tensor_tensor(
                                out=agg[:, ti, :],
                                in0=acc[:],
                                in1=recipb[:, t * P : (t + 1) * P],
                                op=mult,
                            )
                    if mode == "partial":
                        continue

                    hTdst, hoff = hTdst_fn(t0, t1)
                    for b0 in range(t0, t1, DENSE_B):
                        b1 = min(b0 + DENSE_B, t1)
                        w = (b1 - b0) * P
                        hps = ps_h.tile([P, DENSE_B * P], f32, tag="h")
                        nc.tensor.matmul(
                            out=hps[:, :w], lhsT=wl[:],
                            rhs=agg[:, b0 - t0 : b1 - t0, :],
                            start=True, stop=False,
                        )
                        nc.tensor.matmul(
                            out=hps[:, :w], lhsT=wr[:],
                            rhs=selfT[:, soff_base + b0 * P : soff_base + b1 * P],
                            start=False, stop=True,
                        )
                        nc.scalar.activation(
                            out=hTdst[:, hoff + (b0 - t0) * P
                                      : hoff + (b1 - t0) * P],
                            in_=hps[:, :w],
                            func=(
                                mybir.ActivationFunctionType.Relu
                                if relu
                                else mybir.ActivationFunctionType.Identity
                            ),
                            bias=bias[:, 0:1],
                        )

                    rowbuf = rpool.tile([P, nt, P],
                                        row_write(t0, t1, None, probe=True),
                                        tag="row", name="rowbuf")
                    for ti in range(nt):
                        tps = ps_t.tile([P, P], bf16, tag="tp")
                        nc.tensor.transpose(
                            out=tps[:],
                            in_=hTdst[:, hoff + ti * P : hoff + (ti + 1) * P],
                            identity=ident[:],
                        )
                        nc.vector.tensor_copy(out=rowbuf[:, ti, :], in_=tps[:])
                    row_write(t0, t1, rowbuf)

            def hT_resident(t0, t1):
                return hT, t0 * P

            def hT_scratch(t0, t1):
                t = apool.tile([P, (t1 - t0) * P], bf16, tag="hT2", name="hT2")
                return t, 0

            def h_row_write(t0, t1, rowbuf, probe=False):
                if probe:
                    return mybir.dt.bfloat16
                # split at the A/B boundary tile
                parts = []
                if t0 < TSPL:
                    e = min(t1, TSPL)
                    parts.append((hbA, t0 * P, e * P, 0))
                if t1 > TSPL:
                    s = max(t0, TSPL)
                    parts.append((hbB, s * P - NCORES * 0 - ASPL,
                                  t1 * P - ASPL, s - t0))
                for (dst, r0, r1, toff) in parts:
                    nrows = r1 - r0
                    nc.sync.dma_start(
                        out=dst[r0:r1, :].rearrange("(t p) q -> p t q", p=P),
                        in_=rowbuf[:, toff : toff + nrows // P, :],
                    )

            def out_row_write(t0, t1, rowbuf, probe=False):
                if probe:
                    return mybir.dt.float32
                nc.sync.dma_start(
                    out=out_d[t0 * P : t1 * P, :].rearrange(
                        "(t p) q -> p t q", p=P
                    ),
                    in_=rowbuf[:],
                )

            xgA = xg_d[0:NA, :]
            xgB = xg_d[NA : NA + NB, :]
            layer((xgA, xgB), [0, 1, 2, 3], "single", xownT, 0, wl1, wr1,
                  bias1, True, hT_resident, h_row_write,
                  tcols_fn=lambda t: tile_cols[t])

            nc.gpsimd.collective_compute(
                "AllGather",
                mybir.AluOpType.bypass,
                replica_groups=[list(range(NCORES))],
                ins=[hbA[:]],
                outs=[hfA[:]],
            )

            # layer-2 pass A: aggregate A-window chunks into partial sums
            # (aggP reuses xownT's buffer once layer 1 is done with it)
            aggP = bigpool.tile([P, NPC_PAD], bf16, tag="big", name="aggP")
            layer((hfA2, hfB2), [0, 1], "partial", hT, 0, wl2, wr2,
                  bias2, False, hT_scratch, out_row_write, aggP=aggP,
                  tcols_fn=lambda t: tile_colsA[t])

            # CC-B sits after pass-A's gathers in Pool program order, so
            # its transfer overlaps pass-A compute
            nc.gpsimd.collective_compute(
                "AllGather",
                mybir.AluOpType.bypass,
                replica_groups=[list(range(NCORES))],
                ins=[hbB[:]],
                outs=[hfB[:]],
            )

            layer((hfA2, hfB2), [2, 3], "finish", hT, 0, wl2, wr2,
                  bias2, False, hT_scratch, out_row_write, aggP=aggP,
                  tcols_fn=lambda t: tile_colsB[t])

    return nc


def run(x, edge_index, W_l1, b_l1, W_r1, W_l2, b_l2, W_r2, trace=False):
    n_nodes = x.shape[0]
    assert n_nodes == NCORES * NPC

    gmaxc = int(os.environ.get("SAGE_GMAXC", "8"))

    ch, colof, ncols, groups, idx16, edst, recip = _prep_edges(
        np.asarray(edge_index), n_nodes
    )
    tile_cols = [
        [c for r in range(NWIN)
         for c in range(int(colof[t, r]), int(colof[t, r] + ch[t, r]))]
        for t in range(TPC)
    ]
    gcmax = max(g[3] - g[2] for g in groups)

    x = np.asarray(x, np.float32)
    x_pad = np.zeros((NALL, P), np.float32)
    for c in range(NCORES):
        x_pad[c * NPC_PAD : c * NPC_PAD + NPC] = x[c * NPC : (c + 1) * NPC]
    # A/B-relaid gather table
    x_ab = np.empty((NA + NB, P), np.float32)
    for c in range(NCORES):
        sl = x_pad[c * NPC_PAD : (c + 1) * NPC_PAD]
        x_ab[c * ASPL : (c + 1) * ASPL] = sl[:ASPL]
        x_ab[NA + c * BSPL : NA + (c + 1) * BSPL] = sl[ASPL:]

    import ml_dtypes

    bf = ml_dtypes.bfloat16
    common = {
        "xg": x_ab.astype(bf),
        "wl1": np.asarray(W_l1, np.float32).astype(bf),
        "wr1": np.asarray(W_r1, np.float32).astype(bf),
        "wl2": np.asarray(W_l2, np.float32).astype(bf),
        "wr2": np.asarray(W_r2, np.float32).astype(bf),
        "bias1": np.asarray(b_l1, np.float32).reshape(P, 1),
        "bias2": np.asarray(b_l2, np.float32).reshape(P, 1),
        "iota": np.ascontiguousarray(
            np.broadcast_to(np.arange(P, dtype=np.float32), (P, P))
        ).astype(bf),
        "ident": np.eye(P, dtype=np.float32).astype(bf),
    }
    in_maps = []
    for c in range(NCORES):
        m = dict(common)
        m["xownT"] = np.ascontiguousarray(
            x_pad[c * NPC_PAD : (c + 1) * NPC_PAD].T
        ).astype(bf)
        m["idx16"] = idx16[c]
        m["edst"] = edst[c].astype(bf)
        rb = np.zeros(NPC_PAD, np.float32)
        rb[:NPC] = recip[c * NPC : (c + 1) * NPC]
        m["recipb"] = np.ascontiguousarray(
            np.broadcast_to(rb, (P, NPC_PAD))
        ).astype(bf)
        in_maps.append(m)

    tile_colsA = [
        [c for r in (0, 1)
         for c in range(int(colof[t, r]), int(colof[t, r] + ch[t, r]))]
        for t in range(TPC)
    ]
    tile_colsB = [
        [c for r in (2, 3)
         for c in range(int(colof[t, r]), int(colof[t, r] + ch[t, r]))]
        for t in range(TPC)
    ]
    nc = _build_program(ncols, groups, tile_cols, tile_colsA, tile_colsB,
                        gcmax, gmaxc)
    nc.finalize()

    from concourse.bass_utils import run_bass_kernel_spmd

    res = run_bass_kernel_spmd(nc, in_maps, list(range(NCORES)), trace=trace)
    out = np.empty((n_nodes, P), np.float32)
    for c in range(NCORES):
        out[c * NPC : (c + 1) * NPC] = res.results[c]["out"][:NPC]
    return out, res


def kernel(x, edge_index, W_l1, b_l1, W_r1, W_l2, b_l2, W_r2):
    out, _ = run(x, edge_index, W_l1, b_l1, W_r1, W_l2, b_l2, W_r2)
    return out


# revision 15
# speedup vs baseline: 1.0510x; 1.0510x over previous
"""Two-layer GraphSAGE (mean aggregation) on 8 Trainium2 NeuronCores.

Strategy (dst-partitioning per the hint):
- Nodes partitioned by destination across 8 cores (12500 each, padded to
  12544 = 98*128 rows). Each core owns edges whose dst is in its slice,
  bucketed on host by (dst tile, src window); 4 windows of <=32768 rows
  make gather indices fit int16.
- x[src] rows are fetched in bf16 with batched SWDGE dma_gather (1024
  descriptors per instruction — the ucode ring limit — rotating across
  the 4 SWDGE queues so descriptor generation pipelines with transfers).
- Aggregation per 128-dst tile: indicator matmuls on the PE. Indicators
  (pure 0/1) are built in one DVE is_equal per (group, window) span; the
  1/deg mean scaling is applied on the PSUM drain via a host-shipped
  [128, NPC_PAD] broadcast reciprocal table.
- Downstream stays in transposed [feat, node] layout: hT = W_l^T @ aggT
  + W_r^T @ xT, so bias+relu+PSUM-drain is a single Activation op and
  layer-2's self term reuses layer-1's resident hT tile. Row-layout h
  (gather table / final output) is produced with PE transposes and one
  batched DMA per tile-group.
- Between layers the bf16 h slices are AllGathered.

kernel(**inputs) -> np.ndarray takes FULL inputs, returns FULL output.
"""

import os

import numpy as np

P = 128
NCORES = 8
NPC = 12500            # nodes per core
TPC = 98               # 128-node tiles per core
NPC_PAD = TPC * P      # 12544
NALL = NCORES * NPC_PAD  # 100352
NWIN = 4
WIN = 32768            # window size so gather indices fit int16
WLIM = [min((w + 1) * WIN, NALL) for w in range(NWIN)]
WBASE = [w * WIN for w in range(NWIN)]
TPG = 12               # tiles per group
DENSE_B = 4            # tiles per dense-matmul batch (512 moving free dim)


def _prep_edges(edge_index: np.ndarray, n_nodes: int):
    """Bucket edges by (owner core, dst tile, src window).

    Column layout (shared by all cores; chunk counts are max over cores so
    the SPMD program is uniform): group-major, then window, then tile.
    """
    src = edge_index[0].astype(np.int64)
    dst = edge_index[1].astype(np.int64)
    srcpad = (src // NPC) * NPC_PAD + (src % NPC)
    rng = srcpad >> 15  # window index, 0..3
    core = dst // NPC
    loc = dst % NPC
    tl = loc // P
    off = loc % P

    key = (core * TPC + tl) * NWIN + rng
    cnt = np.bincount(key, minlength=NCORES * TPC * NWIN).reshape(
        NCORES, TPC, NWIN
    )
    ch = -(-cnt.max(axis=0) // P)  # [TPC, NWIN], 0 allowed
    assert ch.sum(axis=1).min() >= 1

    colof = np.zeros((TPC, NWIN), np.int64)
    groups = []  # (t0, t1, gc0, gc1, spans[r] = (c0, c1))
    c = 0
    for g0 in range(0, TPC, TPG):
        t0, t1 = g0, min(g0 + TPG, TPC)
        gc0 = c
        spans = []
        for r in range(NWIN):
            rc0 = c
            for t in range(t0, t1):
                colof[t, r] = c
                c += ch[t, r]
            spans.append((rc0, c))
        groups.append((t0, t1, gc0, c, spans))
    ncols = int(c)
    S = ncols * P

    # rank of each edge within its (core, tile, window) bucket
    order = np.argsort(key, kind="stable")
    sk = key[order]
    first = np.r_[True, sk[1:] != sk[:-1]]
    idx_of_first = np.where(first)[0]
    grp_id = np.cumsum(first) - 1
    rank = np.arange(len(sk)) - idx_of_first[grp_id]
    slot = colof[tl[order], rng[order]] * P + rank

    idxflat = np.zeros((NCORES, S), np.int16)
    edstflat = np.full((NCORES, S), -1.0, np.float32)
    idxval = (srcpad - np.asarray(WBASE, np.int64)[rng]).astype(np.int16)
    co = core[order]
    idxflat[co, slot] = idxval[order]
    edstflat[co, slot] = off[order].astype(np.float32)

    deg = np.bincount(dst, minlength=n_nodes).astype(np.float64)
    recip = (1.0 / np.maximum(deg, 1.0)).astype(np.float32)  # [n_nodes]

    idx16 = np.ascontiguousarray(
        np.tile(idxflat.reshape(NCORES, S // 16, 16).transpose(0, 2, 1), (1, 8, 1))
    )
    edst = np.ascontiguousarray(
        edstflat.reshape(NCORES, ncols, P).transpose(0, 2, 1)
    )
    return ch, colof, ncols, groups, idx16, edst, recip


def _build_program(ncols, groups, tile_cols, gcmax, gmaxc):
    from concourse import bacc, library_config, mybir, tile

    f32 = mybir.dt.float32
    bf16 = mybir.dt.bfloat16
    i16 = mybir.dt.int16

    nc = bacc.Bacc(
        "TRN2",
        target_bir_lowering=False,
        debug=False,
        num_devices=NCORES,
        num_swdge_queues=4,
    )

    xg_d = nc.declare_dram_parameter("xg", [NALL, P], bf16, isOutput=False)
    xownT_d = nc.declare_dram_parameter("xownT", [P, NPC_PAD], bf16, isOutput=False)
    idx_d = nc.declare_dram_parameter("idx16", [P, (ncols * P) // 16], i16,
                                      isOutput=False)
    edst_d = nc.declare_dram_parameter("edst", [P, ncols], f32, isOutput=False)
    recipb_d = nc.declare_dram_parameter("recipb", [P, NPC_PAD], bf16,
                                         isOutput=False)
    wl1_d = nc.declare_dram_parameter("wl1", [P, P], bf16, isOutput=False)
    wr1_d = nc.declare_dram_parameter("wr1", [P, P], bf16, isOutput=False)
    wl2_d = nc.declare_dram_parameter("wl2", [P, P], bf16, isOutput=False)
    wr2_d = nc.declare_dram_parameter("wr2", [P, P], bf16, isOutput=False)
    bias1_d = nc.declare_dram_parameter("bias1", [P, 1], f32, isOutput=False)
    bias2_d = nc.declare_dram_parameter("bias2", [P, 1], f32, isOutput=False)
    iota_d = nc.declare_dram_parameter("iota", [P, P], bf16, isOutput=False)
    ident_d = nc.declare_dram_parameter("ident", [P, P], bf16, isOutput=False)
    out_d = nc.declare_dram_parameter("out", [NPC_PAD, P], f32, isOutput=True)

    is_eq = mybir.AluOpType.is_equal
    mult = mybir.AluOpType.mult

    with tile.TileContext(nc) as tc:
        with (
            tc.tile_pool(name="const", bufs=1) as cpool,
            tc.tile_pool(name="gath", bufs=2) as gpool,
            tc.tile_pool(name="indp", bufs=2) as ipool,
            tc.tile_pool(name="agg", bufs=2) as apool,
            tc.tile_pool(name="row", bufs=2) as rpool,
            tc.tile_pool(name="psacc", bufs=2, space="PSUM") as ps_a,
            tc.tile_pool(name="psh", bufs=2, space="PSUM") as ps_h,
            tc.tile_pool(name="pst", bufs=2, space="PSUM") as ps_t,
            tc.tile_pool(name="dram", bufs=1, space="DRAM") as dpool,
        ):
            # InstDMAGatherAnt lives in the mlp Q7 ucode library
            nc.gpsimd.load_library(library_config.mlp)

            def load_const(dram_ap, shape, dtype, name):
                t = cpool.tile(shape, dtype, name=name)
                nc.sync.dma_start(out=t[:], in_=dram_ap)
                return t

            wl1 = load_const(wl1_d[:], [P, P], bf16, "wl1")
            wr1 = load_const(wr1_d[:], [P, P], bf16, "wr1")
            wl2 = load_const(wl2_d[:], [P, P], bf16, "wl2")
            wr2 = load_const(wr2_d[:], [P, P], bf16, "wr2")
            bias1 = load_const(bias1_d[:], [P, 1], f32, "bias1")
            bias2 = load_const(bias2_d[:], [P, 1], f32, "bias2")
            iota = load_const(iota_d[:], [P, P], bf16, "iota")
            ident = load_const(ident_d[:], [P, P], bf16, "ident")
            xownT = load_const(xownT_d[:], [P, NPC_PAD], bf16, "xownT")
            idx16 = load_const(idx_d[:], [P, (ncols * P) // 16], i16, "idx16")
            edst = load_const(edst_d[:], [P, ncols], f32, "edst")
            recipb = load_const(recipb_d[:], [P, NPC_PAD], bf16, "recipb")

            hT = cpool.tile([P, NPC_PAD], bf16, name="hT")

            h_bounce = dpool.tile([NPC_PAD, P], bf16, name="h_bounce")
            h_full3 = dpool.tile(
                [NCORES, NPC_PAD, P], bf16, name="h_full", addr_space="Shared"
            )
            h_full2 = h_full3[:].rearrange("c n d -> (c n) d")

            qctr = [0]

            def layer(src2d, selfT, wl, wr, bias, relu, hTdst_fn, rowdst,
                      rowdtype):
                for gi, (t0, t1, gc0, gc1, spans) in enumerate(groups):
                    nt = t1 - t0
                    gbuf = gpool.tile([P, gcmax, P], bf16, tag="g")
                    ibuf = ipool.tile([P, gcmax, P], bf16, tag="i")
                    for r, (c0, c1) in enumerate(spans):
                        if c1 == c0:
                            continue
                        for s0 in range(c0, c1, gmaxc):
                            s1 = min(s0 + gmaxc, c1)
                            n_idx = (s1 - s0) * P
                            nc.gpsimd.dma_gather(
                                gbuf[:, s0 - gc0 : s1 - gc0, :],
                                src2d[WBASE[r] : WLIM[r], :],
                                idx16[:, s0 * 8 : s1 * 8],
                                n_idx,
                                n_idx,
                                P,
                                queue_num=qctr[0] % 4,
                            )
                            qctr[0] += 1
                        # 0/1 indicator for the whole span in one DVE op
                        nc.vector.tensor_tensor(
                            out=ibuf[:, c0 - gc0 : c1 - gc0, :],
                            in0=edst[:, c0:c1, None].to_broadcast(
                                [P, c1 - c0, P]
                            ),
                            in1=iota[:, None, :].to_broadcast([P, c1 - c0, P]),
                            op=is_eq,
                        )
                    agg = apool.tile([P, nt, P], bf16, tag="agg")
                    for ti, t in enumerate(range(t0, t1)):
                        cols = tile_cols[t]
                        acc = ps_a.tile([P, P], f32, tag="acc")
                        last = len(cols) - 1
                        for ci, col in enumerate(cols):
                            nc.tensor.matmul(
                                out=acc[:],
                                lhsT=gbuf[:, col - gc0, :],
                                rhs=ibuf[:, col - gc0, :],
                                start=(ci == 0),
                                stop=(ci == last),
                            )
                        # drain PSUM with the 1/deg mean scaling fused in
                        nc.vector.tensor_tensor(
                            out=agg[:, ti, :],
                            in0=acc[:],
                            in1=recipb[:, t * P : (t + 1) * P],
                            op=mult,
                        )

                    hTdst, hoff = hTdst_fn(t0, t1)
                    for b0 in range(t0, t1, DENSE_B):
                        b1 = min(b0 + DENSE_B, t1)
                        w = (b1 - b0) * P
                        hps = ps_h.tile([P, DENSE_B * P], f32, tag="h")
                        nc.tensor.matmul(
                            out=hps[:, :w], lhsT=wl[:],
                            rhs=agg[:, b0 - t0 : b1 - t0, :],
                            start=True, stop=False,
                        )
                        nc.tensor.matmul(
                            out=hps[:, :w], lhsT=wr[:],
                            rhs=selfT[:, b0 * P : b1 * P],
                            start=False, stop=True,
                        )
                        nc.scalar.activation(
                            out=hTdst[:, hoff + (b0 - t0) * P
                                      : hoff + (b1 - t0) * P],
                            in_=hps[:, :w],
                            func=(
                                mybir.ActivationFunctionType.Relu
                                if relu
                                else mybir.ActivationFunctionType.Identity
                            ),
                            bias=bias[:, 0:1],
                        )

                    rowbuf = rpool.tile([P, nt, P], rowdtype,
                                        tag=f"row{rowdtype}", name="rowbuf")
                    for ti in range(nt):
                        tps = ps_t.tile([P, P], bf16, tag="tp")
                        nc.tensor.transpose(
                            out=tps[:],
                            in_=hTdst[:, hoff + ti * P : hoff + (ti + 1) * P],
                            identity=ident[:],
                        )
                        nc.vector.tensor_copy(out=rowbuf[:, ti, :], in_=tps[:])
                    nc.sync.dma_start(
                        out=rowdst[t0 * P : t1 * P, :].rearrange(
                            "(t p) q -> p t q", p=P
                        ),
                        in_=rowbuf[:],
                    )

            def hT_resident(t0, t1):
                return hT, t0 * P

            def hT_scratch(t0, t1):
                t = apool.tile([P, (t1 - t0) * P], bf16, tag="hT2", name="hT2")
                return t, 0

            layer(xg_d, xownT, wl1, wr1, bias1, True, hT_resident,
                  h_bounce, mybir.dt.bfloat16)

            nc.gpsimd.collective_compute(
                "AllGather",
                mybir.AluOpType.bypass,
                replica_groups=[list(range(NCORES))],
                ins=[h_bounce[:]],
                outs=[h_full3[:]],
            )

            layer(h_full2, hT, wl2, wr2, bias2, False, hT_scratch,
                  out_d, mybir.dt.float32)

    return nc


def run(x, edge_index, W_l1, b_l1, W_r1, W_l2, b_l2, W_r2, trace=False):
    n_nodes = x.shape[0]
    assert n_nodes == NCORES * NPC

    gmaxc = int(os.environ.get("SAGE_GMAXC", "8"))  # cols per dma_gather

    ch, colof, ncols, groups, idx16, edst, recip = _prep_edges(
        np.asarray(edge_index), n_nodes
    )
    tile_cols = [
        [c for r in range(NWIN)
         for c in range(int(colof[t, r]), int(colof[t, r] + ch[t, r]))]
        for t in range(TPC)
    ]
    gcmax = max(g[3] - g[2] for g in groups)

    x = np.asarray(x, np.float32)
    x_pad = np.zeros((NALL, P), np.float32)
    for c in range(NCORES):
        x_pad[c * NPC_PAD : c * NPC_PAD + NPC] = x[c * NPC : (c + 1) * NPC]

    import ml_dtypes

    bf = ml_dtypes.bfloat16
    common = {
        "xg": x_pad.astype(bf),
        "wl1": np.asarray(W_l1, np.float32).astype(bf),
        "wr1": np.asarray(W_r1, np.float32).astype(bf),
        "wl2": np.asarray(W_l2, np.float32).astype(bf),
        "wr2": np.asarray(W_r2, np.float32).astype(bf),
        "bias1": np.asarray(b_l1, np.float32).reshape(P, 1),
        "bias2": np.asarray(b_l2, np.float32).reshape(P, 1),
        "iota": np.ascontiguousarray(
            np.broadcast_to(np.arange(P, dtype=np.float32), (P, P))
        ).astype(bf),
        "ident": np.eye(P, dtype=np.float32).astype(bf),
    }
    in_maps = []
    for c in range(NCORES):
        m = dict(common)
        m["xownT"] = np.ascontiguousarray(
            x_pad[c * NPC_PAD : (c + 1) * NPC_PAD].T
        ).astype(bf)
        m["idx16"] = idx16[c]
        m["edst"] = edst[c]
        rb = np.zeros(NPC_PAD, np.float32)
        rb[:NPC] = recip[c * NPC : (c + 1) * NPC]
        m["recipb"] = np.ascontiguousarray(
            np.broadcast_to(rb, (P, NPC_PAD))
        ).astype(bf)
        in_maps.append(m)

    nc = _build_program(ncols, groups, tile_cols, gcmax, gmaxc)
    nc.finalize()

    from concourse.bass_utils import run_bass_kernel_spmd

    res = run_bass_kernel_spmd(nc, in_maps, list(range(NCORES)), trace=trace)
    out = np.empty((n_nodes, P), np.float32)
    for c in range(NCORES):
        out[c * NPC : (c + 1) * NPC] = res.results[c]["out"][:NPC]
    return out, res


def kernel(x, edge_index, W_l1, b_l1, W_r1, W_l2, b_l2, W_r2):
    out, _ = run(x, edge_index, W_l1, b_l1, W_r1, W_l2, b_l2, W_r2)
    return out


# revision 16
# speedup vs baseline: 1.2174x; 1.1583x over previous
"""Two-layer GraphSAGE (mean aggregation) on 8 Trainium2 NeuronCores.

Strategy (dst-partitioning per the hint):
- Nodes partitioned by destination across 8 cores (12500 each, padded to
  12544 = 98*128 rows). Each core owns edges whose dst is in its slice,
  bucketed on host by (dst tile, src window); 4 windows of <=32768 rows
  make gather indices fit int16.
- x[src] rows are fetched in bf16 with batched SWDGE dma_gather (1024
  descriptors per instruction — the ucode ring limit — rotating across
  the 4 SWDGE queues so descriptor generation pipelines with transfers).
- Aggregation per 128-dst tile: indicator matmuls on the PE. Indicators
  (pure 0/1) are built in one DVE is_equal per (group, window) span; the
  1/deg mean scaling is applied on the PSUM drain via a host-shipped
  [128, NPC_PAD] broadcast reciprocal table.
- Downstream stays in transposed [feat, node] layout: hT = W_l^T @ aggT
  + W_r^T @ xT, so bias+relu+PSUM-drain is a single Activation op and
  layer-2's self term reuses layer-1's resident hT tile. Row-layout h
  (gather table / final output) is produced with PE transposes and one
  batched DMA per tile-group.
- Between layers the bf16 h slices are AllGathered.

kernel(**inputs) -> np.ndarray takes FULL inputs, returns FULL output.
"""

import os

import numpy as np

P = 128
NCORES = 8
NPC = 12500            # nodes per core
TPC = 98               # 128-node tiles per core
NPC_PAD = TPC * P      # 12544
NALL = NCORES * NPC_PAD  # 100352
NWIN = 4
WIN = 32768            # window size so gather indices fit int16
WLIM = [min((w + 1) * WIN, NALL) for w in range(NWIN)]
WBASE = [w * WIN for w in range(NWIN)]
TPG = 12               # tiles per group
DENSE_B = 4            # tiles per dense-matmul batch (512 moving free dim)


def _prep_edges(edge_index: np.ndarray, n_nodes: int):
    """Bucket edges by (owner core, dst tile, src window).

    Column layout (shared by all cores; chunk counts are max over cores so
    the SPMD program is uniform): group-major, then window, then tile.
    """
    src = edge_index[0].astype(np.int64)
    dst = edge_index[1].astype(np.int64)
    srcpad = (src // NPC) * NPC_PAD + (src % NPC)
    rng = srcpad >> 15  # window index, 0..3
    core = dst // NPC
    loc = dst % NPC
    tl = loc // P
    off = loc % P

    key = (core * TPC + tl) * NWIN + rng
    cnt = np.bincount(key, minlength=NCORES * TPC * NWIN).reshape(
        NCORES, TPC, NWIN
    )
    ch = -(-cnt.max(axis=0) // P)  # [TPC, NWIN], 0 allowed
    assert ch.sum(axis=1).min() >= 1

    colof = np.zeros((TPC, NWIN), np.int64)
    groups = []  # (t0, t1, gc0, gc1, spans[r] = (c0, c1))
    c = 0
    for g0 in range(0, TPC, TPG):
        t0, t1 = g0, min(g0 + TPG, TPC)
        gc0 = c
        spans = []
        for r in range(NWIN):
            rc0 = c
            for t in range(t0, t1):
                colof[t, r] = c
                c += ch[t, r]
            spans.append((rc0, c))
        groups.append((t0, t1, gc0, c, spans))
    ncols = int(c)
    S = ncols * P

    # rank of each edge within its (core, tile, window) bucket
    order = np.argsort(key, kind="stable")
    sk = key[order]
    first = np.r_[True, sk[1:] != sk[:-1]]
    idx_of_first = np.where(first)[0]
    grp_id = np.cumsum(first) - 1
    rank = np.arange(len(sk)) - idx_of_first[grp_id]
    slot = colof[tl[order], rng[order]] * P + rank

    idxflat = np.zeros((NCORES, S), np.int16)
    edstflat = np.full((NCORES, S), -1.0, np.float32)
    idxval = (srcpad - np.asarray(WBASE, np.int64)[rng]).astype(np.int16)
    co = core[order]
    idxflat[co, slot] = idxval[order]
    edstflat[co, slot] = off[order].astype(np.float32)

    deg = np.bincount(dst, minlength=n_nodes).astype(np.float64)
    recip = (1.0 / np.maximum(deg, 1.0)).astype(np.float32)  # [n_nodes]

    idx16 = np.ascontiguousarray(
        np.tile(idxflat.reshape(NCORES, S // 16, 16).transpose(0, 2, 1), (1, 8, 1))
    )
    edst = np.ascontiguousarray(
        edstflat.reshape(NCORES, ncols, P).transpose(0, 2, 1)
    )
    return ch, colof, ncols, groups, idx16, edst, recip


def _build_program(ncols, groups, tile_cols, gcmax, gmaxc):
    from concourse import bacc, library_config, mybir, tile

    f32 = mybir.dt.float32
    bf16 = mybir.dt.bfloat16
    i16 = mybir.dt.int16

    nc = bacc.Bacc(
        "TRN2",
        target_bir_lowering=False,
        debug=False,
        num_devices=NCORES,
        num_swdge_queues=4,
    )

    xg_d = nc.declare_dram_parameter("xg", [NALL, P], bf16, isOutput=False)
    xownT_d = nc.declare_dram_parameter("xownT", [P, NPC_PAD], bf16, isOutput=False)
    idx_d = nc.declare_dram_parameter("idx16", [P, (ncols * P) // 16], i16,
                                      isOutput=False)
    edst_d = nc.declare_dram_parameter("edst", [P, ncols], f32, isOutput=False)
    recipb_d = nc.declare_dram_parameter("recipb", [P, NPC_PAD], bf16,
                                         isOutput=False)
    wl1_d = nc.declare_dram_parameter("wl1", [P, P], bf16, isOutput=False)
    wr1_d = nc.declare_dram_parameter("wr1", [P, P], bf16, isOutput=False)
    wl2_d = nc.declare_dram_parameter("wl2", [P, P], bf16, isOutput=False)
    wr2_d = nc.declare_dram_parameter("wr2", [P, P], bf16, isOutput=False)
    bias1_d = nc.declare_dram_parameter("bias1", [P, 1], f32, isOutput=False)
    bias2_d = nc.declare_dram_parameter("bias2", [P, 1], f32, isOutput=False)
    iota_d = nc.declare_dram_parameter("iota", [P, P], bf16, isOutput=False)
    ident_d = nc.declare_dram_parameter("ident", [P, P], bf16, isOutput=False)
    out_d = nc.declare_dram_parameter("out", [NPC_PAD, P], f32, isOutput=True)

    is_eq = mybir.AluOpType.is_equal
    mult = mybir.AluOpType.mult

    with tile.TileContext(nc) as tc:
        with (
            tc.tile_pool(name="const", bufs=1) as cpool,
            tc.tile_pool(name="gath", bufs=2) as gpool,
            tc.tile_pool(name="indp", bufs=2) as ipool,
            tc.tile_pool(name="agg", bufs=2) as apool,
            tc.tile_pool(name="row", bufs=2) as rpool,
            tc.tile_pool(name="psacc", bufs=4, space="PSUM") as ps_a,
            tc.tile_pool(name="psh", bufs=2, space="PSUM") as ps_h,
            tc.tile_pool(name="pst", bufs=2, space="PSUM") as ps_t,
            tc.tile_pool(name="dram", bufs=1, space="DRAM") as dpool,
        ):
            # InstDMAGatherAnt lives in the mlp Q7 ucode library
            nc.gpsimd.load_library(library_config.mlp)

            def load_const(dram_ap, shape, dtype, name):
                t = cpool.tile(shape, dtype, name=name)
                nc.sync.dma_start(out=t[:], in_=dram_ap)
                return t

            wl1 = load_const(wl1_d[:], [P, P], bf16, "wl1")
            wr1 = load_const(wr1_d[:], [P, P], bf16, "wr1")
            wl2 = load_const(wl2_d[:], [P, P], bf16, "wl2")
            wr2 = load_const(wr2_d[:], [P, P], bf16, "wr2")
            bias1 = load_const(bias1_d[:], [P, 1], f32, "bias1")
            bias2 = load_const(bias2_d[:], [P, 1], f32, "bias2")
            iota = load_const(iota_d[:], [P, P], bf16, "iota")
            ident = load_const(ident_d[:], [P, P], bf16, "ident")
            xownT = load_const(xownT_d[:], [P, NPC_PAD], bf16, "xownT")
            idx16 = load_const(idx_d[:], [P, (ncols * P) // 16], i16, "idx16")
            edst = load_const(edst_d[:], [P, ncols], f32, "edst")
            recipb = load_const(recipb_d[:], [P, NPC_PAD], bf16, "recipb")

            hT = cpool.tile([P, NPC_PAD], bf16, name="hT")

            h_bounce = dpool.tile([NPC_PAD, P], bf16, name="h_bounce")
            h_full3 = dpool.tile(
                [NCORES, NPC_PAD, P], bf16, name="h_full", addr_space="Shared"
            )
            h_full2 = h_full3[:].rearrange("c n d -> (c n) d")

            qctr = [0]

            def layer(src2d, selfT, wl, wr, bias, relu, hTdst_fn, rowdst,
                      rowdtype):
                for gi, (t0, t1, gc0, gc1, spans) in enumerate(groups):
                    nt = t1 - t0
                    gbuf = gpool.tile([P, gcmax, P], bf16, tag="g")
                    ibuf = ipool.tile([P, gcmax, P], bf16, tag="i")
                    for r, (c0, c1) in enumerate(spans):
                        if c1 == c0:
                            continue
                        for s0 in range(c0, c1, gmaxc):
                            s1 = min(s0 + gmaxc, c1)
                            n_idx = (s1 - s0) * P
                            nc.gpsimd.dma_gather(
                                gbuf[:, s0 - gc0 : s1 - gc0, :],
                                src2d[WBASE[r] : WLIM[r], :],
                                idx16[:, s0 * 8 : s1 * 8],
                                n_idx,
                                n_idx,
                                P,
                                queue_num=qctr[0] % 4,
                            )
                            qctr[0] += 1
                        # 0/1 indicator for the whole span in one DVE op
                        nc.vector.tensor_tensor(
                            out=ibuf[:, c0 - gc0 : c1 - gc0, :],
                            in0=edst[:, c0:c1, None].to_broadcast(
                                [P, c1 - c0, P]
                            ),
                            in1=iota[:, None, :].to_broadcast([P, c1 - c0, P]),
                            op=is_eq,
                        )
                    agg = apool.tile([P, nt, P], bf16, tag="agg")
                    for ti, t in enumerate(range(t0, t1)):
                        cols = tile_cols[t]
                        acc = ps_a.tile([P, P], f32, tag="acc")
                        last = len(cols) - 1
                        for ci, col in enumerate(cols):
                            nc.tensor.matmul(
                                out=acc[:],
                                lhsT=gbuf[:, col - gc0, :],
                                rhs=ibuf[:, col - gc0, :],
                                start=(ci == 0),
                                stop=(ci == last),
                            )
                        # drain PSUM with the 1/deg mean scaling fused in
                        nc.vector.tensor_tensor(
                            out=agg[:, ti, :],
                            in0=acc[:],
                            in1=recipb[:, t * P : (t + 1) * P],
                            op=mult,
                        )

                    hTdst, hoff = hTdst_fn(t0, t1)
                    for b0 in range(t0, t1, DENSE_B):
                        b1 = min(b0 + DENSE_B, t1)
                        w = (b1 - b0) * P
                        hps = ps_h.tile([P, DENSE_B * P], f32, tag="h")
                        nc.tensor.matmul(
                            out=hps[:, :w], lhsT=wl[:],
                            rhs=agg[:, b0 - t0 : b1 - t0, :],
                            start=True, stop=False,
                        )
                        nc.tensor.matmul(
                            out=hps[:, :w], lhsT=wr[:],
                            rhs=selfT[:, b0 * P : b1 * P],
                            start=False, stop=True,
                        )
                        nc.scalar.activation(
                            out=hTdst[:, hoff + (b0 - t0) * P
                                      : hoff + (b1 - t0) * P],
                            in_=hps[:, :w],
                            func=(
                                mybir.ActivationFunctionType.Relu
                                if relu
                                else mybir.ActivationFunctionType.Identity
                            ),
                            bias=bias[:, 0:1],
                        )

                    rowbuf = rpool.tile([P, nt, P], rowdtype,
                                        tag=f"row{rowdtype}", name="rowbuf")
                    for ti in range(nt):
                        tps = ps_t.tile([P, P], bf16, tag="tp")
                        nc.tensor.transpose(
                            out=tps[:],
                            in_=hTdst[:, hoff + ti * P : hoff + (ti + 1) * P],
                            identity=ident[:],
                        )
                        nc.scalar.copy(out=rowbuf[:, ti, :], in_=tps[:])
                    nc.sync.dma_start(
                        out=rowdst[t0 * P : t1 * P, :].rearrange(
                            "(t p) q -> p t q", p=P
                        ),
                        in_=rowbuf[:],
                    )

            def hT_resident(t0, t1):
                return hT, t0 * P

            def hT_scratch(t0, t1):
                t = apool.tile([P, (t1 - t0) * P], bf16, tag="hT2", name="hT2")
                return t, 0

            layer(xg_d, xownT, wl1, wr1, bias1, True, hT_resident,
                  h_bounce, mybir.dt.bfloat16)

            nc.gpsimd.collective_compute(
                "AllGather",
                mybir.AluOpType.bypass,
                replica_groups=[list(range(NCORES))],
                ins=[h_bounce[:]],
                outs=[h_full3[:]],
            )

            layer(h_full2, hT, wl2, wr2, bias2, False, hT_scratch,
                  out_d, mybir.dt.float32)

    return nc


def run(x, edge_index, W_l1, b_l1, W_r1, W_l2, b_l2, W_r2, trace=False):
    n_nodes = x.shape[0]
    assert n_nodes == NCORES * NPC

    gmaxc = int(os.environ.get("SAGE_GMAXC", "8"))  # cols per dma_gather

    ch, colof, ncols, groups, idx16, edst, recip = _prep_edges(
        np.asarray(edge_index), n_nodes
    )
    tile_cols = [
        [c for r in range(NWIN)
         for c in range(int(colof[t, r]), int(colof[t, r] + ch[t, r]))]
        for t in range(TPC)
    ]
    gcmax = max(g[3] - g[2] for g in groups)

    x = np.asarray(x, np.float32)
    x_pad = np.zeros((NALL, P), np.float32)
    for c in range(NCORES):
        x_pad[c * NPC_PAD : c * NPC_PAD + NPC] = x[c * NPC : (c + 1) * NPC]

    import ml_dtypes

    bf = ml_dtypes.bfloat16
    common = {
        "xg": x_pad.astype(bf),
        "wl1": np.asarray(W_l1, np.float32).astype(bf),
        "wr1": np.asarray(W_r1, np.float32).astype(bf),
        "wl2": np.asarray(W_l2, np.float32).astype(bf),
        "wr2": np.asarray(W_r2, np.float32).astype(bf),
        "bias1": np.asarray(b_l1, np.float32).reshape(P, 1),
        "bias2": np.asarray(b_l2, np.float32).reshape(P, 1),
        "iota": np.ascontiguousarray(
            np.broadcast_to(np.arange(P, dtype=np.float32), (P, P))
        ).astype(bf),
        "ident": np.eye(P, dtype=np.float32).astype(bf),
    }
    in_maps = []
    for c in range(NCORES):
        m = dict(common)
        m["xownT"] = np.ascontiguousarray(
            x_pad[c * NPC_PAD : (c + 1) * NPC_PAD].T
        ).astype(bf)
        m["idx16"] = idx16[c]
        m["edst"] = edst[c]
        rb = np.zeros(NPC_PAD, np.float32)
        rb[:NPC] = recip[c * NPC : (c + 1) * NPC]
        m["recipb"] = np.ascontiguousarray(
            np.broadcast_to(rb, (P, NPC_PAD))
        ).astype(bf)
        in_maps.append(m)

    nc = _build_program(ncols, groups, tile_cols, gcmax, gmaxc)
    nc.finalize()

    from concourse.bass_utils import run_bass_kernel_spmd

    res = run_bass_kernel_spmd(nc, in_maps, list(range(NCORES)), trace=trace)
    out = np.empty((n_nodes, P), np.float32)
    for c in range(NCORES):
        out[c * NPC : (c + 1) * NPC] = res.results[c]["out"][:NPC]
    return out, res


def kernel(x, edge_index, W_l1, b_l1, W_r1, W_l2, b_l2, W_r2):
    out, _ = run(x, edge_index, W_l1, b_l1, W_r1, W_l2, b_l2, W_r2)
    return out
